# revision 1
# baseline (speedup 1.0000x reference)
"""MultiHeadGAT layer on 8 trn2 NeuronCores, data-parallel over batch.

Per core (one batch element):
  Wh = h @ W                                  [1024, 512]  (heads=8, fo=64)
  e_src[h,i], e_dst[h,i] from E = h @ (W @ A)  (WA precomputed on host)
  scores_T[j,i] = leaky_relu(e_src[i] + e_dst[j])   (transposed: j on partitions)
  P = exp(scores_T) * adjT    where exp(leaky(s)) == max(exp(s), exp(0.2 s))
  out[i, h*64+f] = (P.T @ Wh_h)[i,f] / sum_j P[j,i]

AV matmul in transposed orientation: out_T[f,i] = sum_j Wh[j,f]*P[j,i], with a
ones column appended to the lhsT so row 64 of the accumulator is the softmax
denominator.  Engine budget: ACT does the two exps per tile (bias/scale fold
the e_dst add and the 0.2 slope), DVE does max, gpsimd+DVE split the adjacency
mask multiply (adjT kept in bf16 - exact for 0/1 - produced by DMA transpose),
PE does the matmuls fp32.
"""
import sys

sys.path.insert(0, "/opt/trn_rl_repo")

import numpy as np

import concourse.bass as bass
import concourse.mybir as mybir
import concourse.tile as tile
from concourse.bass_utils import run_bass_kernel_spmd
from concourse.masks import make_identity

F32 = mybir.dt.float32
BF16 = mybir.dt.bfloat16
I32 = mybir.dt.int32
AF = mybir.ActivationFunctionType

N_CORES = 8
N = 1024
NB = 8          # row blocks of 128
FIN = 256
KT = 2          # FIN / 128
FO = 512        # heads * fo
H = 8
FOH = 64
ALPHA = 0.2

# tiles whose mask-multiply / max run on gpsimd instead of DVE
GP_MASK_JB = 0  # jb < GP_MASK_JB -> gpsimd handles the mask for that tile

_MAX_SYNC_WAITS = 1


def _split_sync_waits(nc, max_waits=_MAX_SYNC_WAITS):
    """This walrus build rejects instructions carrying more than one sync
    wait; hoist extras onto NOPs inserted just before, on the same engine."""
    uid = 0
    for f in nc.m.functions:
        for bb in f.blocks:
            out = []
            for inst in bb.instructions:
                si = getattr(inst, "sync_info", None)
                if si is not None and si.on_wait and len(si.on_wait) > max_waits:
                    waits = list(si.on_wait)
                    keep = waits[-max_waits:]
                    extra = waits[:-max_waits]
                    si.on_wait.clear()
                    si.on_wait.extend(keep)
                    while extra:
                        chunk, extra = extra[:max_waits], extra[max_waits:]
                        nop = mybir.InstNoOp(
                            name=f"waitsplit-{uid}",
                            engine=inst.engine,
                            sync_info=mybir.SyncInfo(
                                on_wait=list(chunk), on_update=[]
                            ),
                            bass_nofuse=True,
                        )
                        uid += 1
                        out.append(nop)
                out.append(inst)
            bb.instructions[:] = out


def build_nc(split=True):
    nc = bass.Bass()
    h_d = nc.declare_dram_parameter("h", [N, FIN], F32, isOutput=False)
    adj_d = nc.declare_dram_parameter("adj", [N, N], I32, isOutput=False)
    w_d = nc.declare_dram_parameter("W", [FIN, FO], F32, isOutput=False)
    wa_d = nc.declare_dram_parameter("WA", [FIN, 2 * H], F32, isOutput=False)
    out_d = nc.declare_dram_parameter("out", [N, FO], F32, isOutput=True)

    with tile.TileContext(nc) as tc:
        with (
            tc.tile_pool(name="const", bufs=1) as const,
            tc.tile_pool(name="persist", bufs=1) as persist,
            tc.tile_pool(name="ld", bufs=4) as ld,
            tc.tile_pool(name="x1p", bufs=8) as x1p,
            tc.tile_pool(name="x2p", bufs=5) as x2p,
            tc.tile_pool(name="epi", bufs=3) as epi,
            tc.tile_pool(name="psS", bufs=3, space="PSUM") as psS,
            tc.tile_pool(name="psAcc", bufs=2, space="PSUM") as psAcc,
        ):
            ident = const.tile([128, 128], F32, tag="ident")
            make_identity(nc, ident[:])

            wk = []
            for k in range(KT):
                t = const.tile([128, FO], F32, tag=f"W{k}", name=f"W{k}")
                nc.sync.dma_start(t[:], w_d[k * 128:(k + 1) * 128, :])
                wk.append(t)
            wa = []
            for k in range(KT):
                t = const.tile([128, 2 * H], F32, tag=f"WA{k}", name=f"WA{k}")
                nc.sync.dma_start(t[:], wa_d[k * 128:(k + 1) * 128, :])
                wa.append(t)

            # ---- hT[k][f128, i] = h[i, k*128+f] ----
            hT = [persist.tile([128, N], F32, tag=f"hT{k}", name=f"hT{k}")
                  for k in range(KT)]
            for ibq in range(2):      # groups of 4 row-blocks
                hts = []
                for i4 in range(4):
                    ib = ibq * 4 + i4
                    ht = ld.tile([128, FIN], F32, tag="hld")
                    nc.sync.dma_start(ht[:], h_d[ib * 128:(ib + 1) * 128, :])
                    hts.append(ht)
                for k in range(KT):
                    tp = psS.tile([128, 512], F32, tag="ps")
                    for i4 in range(4):
                        nc.tensor.transpose(
                            tp[:, i4 * 128:(i4 + 1) * 128],
                            hts[i4][:, k * 128:(k + 1) * 128], ident[:],
                        )
                    nc.vector.tensor_copy(
                        hT[k][:, ibq * 512:(ibq + 1) * 512], tp[:]
                    )

            # ---- Wh_aug[jb][:, hh*65:+64] = (h @ W) block, col hh*65+64 = 1 ----
            wh_aug = [persist.tile([128, H * 65], F32, tag=f"wha{j}", name=f"wha{j}")
                      for j in range(NB)]
            for jb in range(NB):
                ps = psS.tile([128, 512], F32, tag="ps")
                for k in range(KT):
                    nc.tensor.matmul(
                        ps[:], hT[k][:, jb * 128:(jb + 1) * 128], wk[k][:],
                        start=(k == 0), stop=(k == KT - 1),
                    )
                for hh in range(H):
                    nc.vector.tensor_copy(
                        wh_aug[jb][:, hh * 65:hh * 65 + 64],
                        ps[:, hh * 64:(hh + 1) * 64],
                    )
                for hh in range(H):
                    nc.gpsimd.memset(
                        wh_aug[jb][:, hh * 65 + 64:hh * 65 + 65], 1.0
                    )

            # ---- E_T[16, i] = (WA.T @ hT): rows 0..7 e_src, 8..15 e_dst ----
            e_t = const.tile([16, N], F32, tag="eT")
            for c in range(2):
                ps = psS.tile([16, 512], F32, tag="ps")
                for k in range(KT):
                    nc.tensor.matmul(
                        ps[:], wa[k][:], hT[k][:, c * 512:(c + 1) * 512],
                        start=(k == 0), stop=(k == KT - 1),
                    )
                nc.vector.tensor_copy(e_t[:, c * 512:(c + 1) * 512], ps[:])

            # ---- E[jb][p, 16] = E_T[:, jb*128+p]; e_sc = 0.2 * E ----
            e_sb = [persist.tile([128, 16], F32, tag=f"E{j}", name=f"E{j}")
                    for j in range(NB)]
            e_sc = [persist.tile([128, 16], F32, tag=f"Es{j}", name=f"Es{j}")
                    for j in range(NB)]
            for jb in range(NB):
                tp = psS.tile([128, 512], F32, tag="ps")
                nc.tensor.transpose(
                    tp[:, 0:16], e_t[:, jb * 128:(jb + 1) * 128],
                    ident[0:16, 0:16],
                )
                nc.vector.tensor_copy(e_sb[jb][:], tp[:, 0:16])
                nc.vector.tensor_scalar_mul(e_sc[jb][:], tp[:, 0:16], ALPHA)

            # ---- e_srcb[h][p, i] = e_src[h, i] broadcast over partitions.
            # Heads 0-1 via PE selector matmul (low latency, unblocks the main
            # loop); heads 2-7 via DMA log-doubling (no PE cost, latency
            # hidden behind the first heads' compute). ----
            e_srcb = [persist.tile([128, N], F32, tag=f"esb{hh}", name=f"esb{hh}")
                      for hh in range(H)]
            NSEL = 2
            sel = []
            for hh in range(NSEL):
                t = const.tile([16, 128], F32, tag=f"sel{hh}", name=f"sel{hh}")
                nc.gpsimd.memset(t[:], 0.0)
                # t[p, y] = (p == hh) ? 1.0 : 0.0
                nc.gpsimd.affine_select(
                    out=t[:], in_=t[:], pattern=[[0, 128]],
                    compare_op=mybir.AluOpType.not_equal, fill=1.0,
                    base=-hh, channel_multiplier=1,
                )
                sel.append(t)
            for hh in range(NSEL):
                for c in range(2):
                    ps = psS.tile([128, 512], F32, tag="ps")
                    nc.tensor.matmul(
                        ps[:], sel[hh][:], e_t[:, c * 512:(c + 1) * 512],
                        start=True, stop=True,
                    )
                    nc.vector.tensor_copy(
                        e_srcb[hh][:, c * 512:(c + 1) * 512], ps[:]
                    )
            for hh in range(NSEL, H):
                t = e_srcb[hh]
                nc.sync.dma_start(t[0:1, :], e_t[hh:hh + 1, :])
                p = 1
                while p < 128:
                    nc.sync.dma_start(t[p:2 * p, :], t[0:p, :])
                    p *= 2

            # ---- adjT[jb][j128, i] = adj[i, jb*128+j] as bf16 (PE transpose).
            # jb-major so adjT[0] completes first and unblocks the main loop
            # as early as possible. ----
            identb = const.tile([128, 128], BF16, tag="identb")
            nc.vector.tensor_copy(identb[:], ident[:])
            adjT = [persist.tile([128, N], BF16, tag=f"adjT{j}", name=f"adjT{j}")
                    for j in range(NB)]
            adjf = [persist.tile([128, N], BF16, tag=f"adjf{i}", name=f"adjf{i}")
                    for i in range(NB)]
            for ib in range(NB):
                ai = ld.tile([128, N], I32, tag="adji", bufs=3)
                nc.sync.dma_start(ai[:], adj_d[ib * 128:(ib + 1) * 128, :])
                nc.vector.tensor_copy(adjf[ib][:], ai[:])
            for jb in range(NB):
                for half in range(2):
                    tp = psS.tile([128, 512], BF16, tag="ps")
                    for i4 in range(4):
                        ib = half * 4 + i4
                        nc.tensor.transpose(
                            tp[:, i4 * 128:(i4 + 1) * 128],
                            adjf[ib][:, jb * 128:(jb + 1) * 128],
                            identb[:],
                        )
                    nc.vector.tensor_copy(
                        adjT[jb][:, half * 512:(half + 1) * 512], tp[:]
                    )

            # ---- main attention loop ----
            for hh in range(H):
                acc = [psAcc.tile([65, 512], F32, tag=f"acc{c}", name=f"acc{c}")
                       for c in range(2)]
                for jb in range(NB):
                    x1 = x1p.tile([128, N], F32, tag="x1")
                    nc.scalar.activation(
                        x1[:], e_srcb[hh][:], AF.Exp,
                        bias=e_sb[jb][:, 8 + hh:9 + hh],
                    )
                    x2 = x2p.tile([128, N], F32, tag="x2")
                    nc.scalar.activation(
                        x2[:], e_srcb[hh][:], AF.Exp,
                        bias=e_sc[jb][:, 8 + hh:9 + hh], scale=ALPHA,
                    )
                    nc.vector.tensor_max(x1[:], x1[:], x2[:])
                    if jb < GP_MASK_JB:
                        nc.gpsimd.tensor_mul(x1[:], x1[:], adjT[jb][:])
                    else:
                        nc.vector.tensor_mul(x1[:], x1[:], adjT[jb][:])
                    for c in range(2):
                        nc.tensor.matmul(
                            acc[c][:],
                            wh_aug[jb][:, hh * 65:(hh + 1) * 65],
                            x1[:, c * 512:(c + 1) * 512],
                            start=(jb == 0), stop=(jb == NB - 1),
                        )
                # epilogue: copy acc to SBUF (ACT), transpose back, scale
                acc_sb = epi.tile([65, N], F32, tag="accsb")
                for c in range(2):
                    nc.scalar.copy(acc_sb[:, c * 512:(c + 1) * 512], acc[c][:])
                for c in range(NB):
                    tp = psS.tile([128, 512], F32, tag="ps")
                    nc.tensor.transpose(
                        tp[:, 0:65], acc_sb[:, c * 128:(c + 1) * 128],
                        ident[0:65, 0:65],
                    )
                    rec = epi.tile([128, 1], F32, tag="rec")
                    nc.vector.reciprocal(rec[:], tp[:, 64:65])
                    osm = epi.tile([128, FOH], F32, tag="osm", bufs=4)
                    nc.scalar.activation(
                        osm[:], tp[:, 0:64], AF.Copy, scale=rec[:],
                    )
                    nc.sync.dma_start(
                        out_d[c * 128:(c + 1) * 128,
                              hh * FOH:(hh + 1) * FOH], osm[:],
                    )

    if split:
        _split_sync_waits(nc)
    return nc


_NC_CACHE = None


def _get_nc():
    global _NC_CACHE
    if _NC_CACHE is None:
        _NC_CACHE = build_nc()
    return _NC_CACHE


def _prep_in_maps(h, adj, W, a):
    h = np.ascontiguousarray(h, dtype=np.float32)
    adj = np.ascontiguousarray(adj, dtype=np.int32)
    W = np.ascontiguousarray(W, dtype=np.float32)
    a = np.ascontiguousarray(a, dtype=np.float32)
    amat = np.zeros((FO, 2 * H), dtype=np.float32)
    for hh in range(H):
        amat[hh * FOH:(hh + 1) * FOH, hh] = a[hh, :FOH]
        amat[hh * FOH:(hh + 1) * FOH, H + hh] = a[hh, FOH:]
    wamat = (W @ amat).astype(np.float32)
    return [
        {"h": h[c], "adj": adj[c], "W": W, "WA": wamat}
        for c in range(N_CORES)
    ]


def run(h, adj, W, a, trace=False, **kw):
    nc = _get_nc()
    in_maps = _prep_in_maps(h, adj, W, a)
    res = run_bass_kernel_spmd(nc, in_maps, list(range(N_CORES)), trace=trace, **kw)
    out = np.stack([res.results[c]["out"] for c in range(N_CORES)], axis=0)
    return out.astype(np.float32), res


def kernel(h, adj, W, a):
    out, _ = run(h, adj, W, a)
    return out



# revision 4
# speedup vs baseline: 1.6782x; 1.6782x over previous
"""MultiHeadGAT layer on 8 trn2 NeuronCores, data-parallel over batch.

Per core (one batch element), with softmax-invariant rescaling: dividing the
unnormalized attention P[j,i] = exp(leaky_relu(e_src[i]+e_dst[j])) by
exp(e_src[i]) (a per-i factor that cancels in the softmax) gives

  P'[j,i] = max( exp(-0.8*e_src[i]) * exp(0.2*e_dst[j]),  exp(e_dst[j]) )

i.e. ONE fused DVE tensor_scalar op per [128,1024] tile (mult + max against
two per-partition scalars) in bf16 (4x DVE mode) -- no exps in the main loop
at all.  Mask multiply t *= adjT is a bf16 tensor_tensor (2x mode), split
DVE/gpsimd.  AV matmul in bf16 (1 cycle/row) with a ones column appended to
the lhsT so row 64 of the accumulator is the softmax denominator.

Host-side prep (layout/dtype only): h.T, adj.T as bf16, W and W@A as bf16.
"""
import sys

sys.path.insert(0, "/opt/trn_rl_repo")

import numpy as np
import ml_dtypes

import concourse.bass as bass
import concourse.mybir as mybir
import concourse.tile as tile
from concourse.bass_utils import run_bass_kernel_spmd
from concourse.masks import make_identity

F32 = mybir.dt.float32
BF16 = mybir.dt.bfloat16
AF = mybir.ActivationFunctionType
ALU = mybir.AluOpType
BF16NP = ml_dtypes.bfloat16

N_CORES = 8
N = 1024
NB = 8          # row blocks of 128
FIN = 256
KT = 2          # FIN / 128
FO = 512        # heads * fo
H = 8
FOH = 64
ALPHA = 0.2

# (h, jb) tiles whose mask-multiply runs on gpsimd instead of DVE
def _gp_mask(hh, jb):
    return jb < 2

NSEL = 2        # heads broadcast via PE selector; rest via DMA doubling

_MAX_SYNC_WAITS = 1


def _split_sync_waits(nc, max_waits=_MAX_SYNC_WAITS):
    """This walrus build rejects instructions carrying more than one sync
    wait; hoist extras onto NOPs inserted just before, on the same engine."""
    uid = 0
    for f in nc.m.functions:
        for bb in f.blocks:
            out = []
            for inst in bb.instructions:
                si = getattr(inst, "sync_info", None)
                if si is not None and si.on_wait and len(si.on_wait) > max_waits:
                    waits = list(si.on_wait)
                    keep = waits[-max_waits:]
                    extra = waits[:-max_waits]
                    si.on_wait.clear()
                    si.on_wait.extend(keep)
                    while extra:
                        chunk, extra = extra[:max_waits], extra[max_waits:]
                        nop = mybir.InstNoOp(
                            name=f"waitsplit-{uid}",
                            engine=inst.engine,
                            sync_info=mybir.SyncInfo(
                                on_wait=list(chunk), on_update=[]
                            ),
                            bass_nofuse=True,
                        )
                        uid += 1
                        out.append(nop)
                out.append(inst)
            bb.instructions[:] = out


def build_nc(split=True):
    nc = bass.Bass()
    hT_d = nc.declare_dram_parameter("hTb", [FIN, N], BF16, isOutput=False)
    adjT_d = nc.declare_dram_parameter("adjT", [N, N], BF16, isOutput=False)
    w_d = nc.declare_dram_parameter("Wb", [FIN, FO], BF16, isOutput=False)
    wa_d = nc.declare_dram_parameter("WAb", [FIN, 2 * H], BF16, isOutput=False)
    out_d = nc.declare_dram_parameter("out", [N, FO], F32, isOutput=True)

    with tile.TileContext(nc) as tc:
        with (
            tc.tile_pool(name="const", bufs=1) as const,
            tc.tile_pool(name="persist", bufs=1) as persist,
            tc.tile_pool(name="tp8", bufs=8) as tpool,
            tc.tile_pool(name="epi", bufs=3) as epi,
            tc.tile_pool(name="psS", bufs=3, space="PSUM") as psS,
            tc.tile_pool(name="psAcc", bufs=2, space="PSUM") as psAcc,
        ):
            ident = const.tile([128, 128], F32, tag="ident")
            make_identity(nc, ident[:])

            wk = []
            for k in range(KT):
                t = const.tile([128, FO], BF16, tag=f"W{k}", name=f"W{k}")
                nc.sync.dma_start(t[:], w_d[k * 128:(k + 1) * 128, :])
                wk.append(t)
            wa = []
            for k in range(KT):
                t = const.tile([128, 2 * H], BF16, tag=f"WA{k}", name=f"WA{k}")
                nc.sync.dma_start(t[:], wa_d[k * 128:(k + 1) * 128, :])
                wa.append(t)
            hT = [persist.tile([128, N], BF16, tag=f"hT{k}", name=f"hT{k}")
                  for k in range(KT)]
            for k in range(KT):
                nc.sync.dma_start(hT[k][:], hT_d[k * 128:(k + 1) * 128, :])
            adjT = [persist.tile([128, N], BF16, tag=f"adjT{j}", name=f"adjT{j}")
                    for j in range(NB)]
            for jb in range(NB):
                nc.sync.dma_start(adjT[jb][:], adjT_d[jb * 128:(jb + 1) * 128, :])

            # ---- E_T[16, i] = (WA.T @ hT): rows 0..7 e_src, 8..15 e_dst ----
            e_t = const.tile([16, N], F32, tag="eT")
            for c in range(2):
                ps = psS.tile([16, 512], F32, tag="ps")
                for k in range(KT):
                    nc.tensor.matmul(
                        ps[:], wa[k][:], hT[k][:, c * 512:(c + 1) * 512],
                        start=(k == 0), stop=(k == KT - 1),
                    )
                nc.vector.tensor_copy(e_t[:, c * 512:(c + 1) * 512], ps[:])

            # ---- e_sb[jb][p, 16] = E_T[:, jb*128+p]; s0/s1 = per-j scalars ----
            e_sb = [persist.tile([128, 16], F32, tag=f"E{j}", name=f"E{j}")
                    for j in range(NB)]
            s0sb = [persist.tile([128, H], F32, tag=f"s0{j}", name=f"s0{j}")
                    for j in range(NB)]
            s1sb = [persist.tile([128, H], F32, tag=f"s1{j}", name=f"s1{j}")
                    for j in range(NB)]
            for jb in range(NB):
                tp = psS.tile([128, 512], F32, tag="ps")
                nc.tensor.transpose(
                    tp[:, 0:16], e_t[:, jb * 128:(jb + 1) * 128],
                    ident[0:16, 0:16],
                )
                nc.vector.tensor_copy(e_sb[jb][:], tp[:, 0:16])
                # s0 = exp(alpha * e_dst), s1 = exp(e_dst)
                nc.scalar.activation(
                    s0sb[jb][:], e_sb[jb][:, 8:16], AF.Exp, scale=ALPHA,
                )
                nc.scalar.activation(
                    s1sb[jb][:], e_sb[jb][:, 8:16], AF.Exp, scale=1.0,
                )

            # ---- G8[h, i] = exp(-(1-alpha) * e_src[h, i]) as bf16 ----
            g8 = const.tile([8, N], BF16, tag="g8")
            nc.scalar.activation(g8[:], e_t[0:8, :], AF.Exp, scale=-(1.0 - ALPHA))

            # ---- Gb[h][p, i] = G8[h, i] broadcast over partitions.
            # Heads 0..NSEL-1 via PE selector matmul (low latency); rest via
            # DMA log-doubling. ----
            gb = [persist.tile([128, N], BF16, tag=f"gb{hh}", name=f"gb{hh}")
                  for hh in range(H)]
            sel = []
            for hh in range(NSEL):
                t = const.tile([8, 128], BF16, tag=f"sel{hh}", name=f"sel{hh}")
                nc.gpsimd.memset(t[:], 0.0)
                nc.gpsimd.affine_select(
                    out=t[:], in_=t[:], pattern=[[0, 128]],
                    compare_op=ALU.not_equal, fill=1.0,
                    base=-hh, channel_multiplier=1,
                )
                sel.append(t)
            for hh in range(NSEL):
                for c in range(2):
                    ps = psS.tile([128, 512], F32, tag="ps")
                    nc.tensor.matmul(
                        ps[:], sel[hh][:], g8[:, c * 512:(c + 1) * 512],
                        start=True, stop=True,
                    )
                    nc.scalar.copy(
                        gb[hh][:, c * 512:(c + 1) * 512], ps[:]
                    )
            for hh in range(NSEL, H):
                t = gb[hh]
                nc.sync.dma_start(t[0:1, :], g8[hh:hh + 1, :])
                p = 1
                while p < 128:
                    nc.sync.dma_start(t[p:2 * p, :], t[0:p, :])
                    p *= 2

            # ---- wh_aug[jb][j, h, 0:64] = (h @ W) block bf16, [:, h, 64] = 1 ----
            wh_aug = [persist.tile([128, H, 65], BF16, tag=f"wha{j}",
                                   name=f"wha{j}")
                      for j in range(NB)]
            for jb in range(NB):
                ps = psS.tile([128, H, FOH], F32, tag="ps")
                for k in range(KT):
                    nc.tensor.matmul(
                        ps[:, :, :], hT[k][:, jb * 128:(jb + 1) * 128], wk[k][:],
                        start=(k == 0), stop=(k == KT - 1),
                    )
                nc.scalar.activation(
                    wh_aug[jb][:, :, 0:64], ps[:, :, :], AF.Copy,
                )
                nc.gpsimd.memset(wh_aug[jb][:, :, 64:65], 1.0)

            # ---- main attention loop ----
            for hh in range(H):
                acc = [psAcc.tile([65, 512], F32, tag=f"acc{c}", name=f"acc{c}")
                       for c in range(2)]
                for jb in range(NB):
                    t = tpool.tile([128, N], BF16, tag="t")
                    nc.vector.tensor_scalar(
                        t[:], gb[hh][:],
                        s0sb[jb][:, hh:hh + 1], s1sb[jb][:, hh:hh + 1],
                        ALU.mult, ALU.max,
                    )
                    if _gp_mask(hh, jb):
                        nc.gpsimd.tensor_mul(t[:], t[:], adjT[jb][:])
                    else:
                        nc.vector.tensor_mul(t[:], t[:], adjT[jb][:])
                    for c in range(2):
                        nc.tensor.matmul(
                            acc[c][:],
                            wh_aug[jb][:, hh, :],
                            t[:, c * 512:(c + 1) * 512],
                            start=(jb == 0), stop=(jb == NB - 1),
                        )
                # epilogue: acc -> SBUF (ACT), transpose back, divide, store
                acc_sb = epi.tile([65, N], F32, tag="accsb")
                for c in range(2):
                    nc.scalar.copy(acc_sb[:, c * 512:(c + 1) * 512], acc[c][:])
                rec8 = epi.tile([128, 8], F32, tag="rec8")
                tps = []
                for q in range(2):
                    tp = psS.tile([128, 4 * 65], F32, tag="ps")
                    for r in range(4):
                        cb = q * 4 + r
                        nc.tensor.transpose(
                            tp[:, r * 65:r * 65 + 65],
                            acc_sb[:, cb * 128:(cb + 1) * 128],
                            ident[0:65, 0:65],
                        )
                    nc.vector.reciprocal(
                        rec8[:, q * 4:(q + 1) * 4], tp[:, 64::65]
                    )
                    tps.append(tp)
                for cb in range(NB):
                    q, r = divmod(cb, 4)
                    osm = epi.tile([128, FOH], F32, tag="osm", bufs=4)
                    nc.scalar.activation(
                        osm[:], tps[q][:, r * 65:r * 65 + 64], AF.Copy,
                        scale=rec8[:, cb:cb + 1],
                    )
                    nc.sync.dma_start(
                        out_d[cb * 128:(cb + 1) * 128,
                              hh * FOH:(hh + 1) * FOH], osm[:],
                    )

    if split:
        _split_sync_waits(nc)
    return nc


_NC_CACHE = None


def _get_nc():
    global _NC_CACHE
    if _NC_CACHE is None:
        _NC_CACHE = build_nc()
    return _NC_CACHE


def _prep_in_maps(h, adj, W, a):
    h = np.ascontiguousarray(h, dtype=np.float32)
    adj = np.ascontiguousarray(adj, dtype=np.int32)
    W = np.ascontiguousarray(W, dtype=np.float32)
    a = np.ascontiguousarray(a, dtype=np.float32)
    amat = np.zeros((FO, 2 * H), dtype=np.float32)
    for hh in range(H):
        amat[hh * FOH:(hh + 1) * FOH, hh] = a[hh, :FOH]
        amat[hh * FOH:(hh + 1) * FOH, H + hh] = a[hh, FOH:]
    wamat = (W @ amat).astype(BF16NP)
    wb = W.astype(BF16NP)
    return [
        {
            "hTb": np.ascontiguousarray(h[c].T).astype(BF16NP),
            "adjT": np.ascontiguousarray(adj[c].T).astype(BF16NP),
            "Wb": wb,
            "WAb": wamat,
        }
        for c in range(N_CORES)
    ]


def run(h, adj, W, a, trace=False, **kw):
    nc = _get_nc()
    in_maps = _prep_in_maps(h, adj, W, a)
    res = run_bass_kernel_spmd(nc, in_maps, list(range(N_CORES)), trace=trace, **kw)
    out = np.stack([res.results[c]["out"] for c in range(N_CORES)], axis=0)
    return out.astype(np.float32), res


def kernel(h, adj, W, a):
    out, _ = run(h, adj, W, a)
    return out


# revision 5
# speedup vs baseline: 1.8926x; 1.1278x over previous
"""MultiHeadGAT layer on 8 trn2 NeuronCores, data-parallel over batch.

Per core (one batch element), with softmax-invariant rescaling: dividing the
unnormalized attention P[j,i] = exp(leaky_relu(e_src[i]+e_dst[j])) by
exp(e_src[i]) (a per-i factor that cancels in the softmax) gives

  P'[j,i] = max( exp(-0.8*e_src[i]) * exp(0.2*e_dst[j]),  exp(e_dst[j]) )

i.e. ONE fused DVE tensor_scalar op per [128,1024] tile (mult + max against
two per-partition scalars) in bf16 -- no exps in the main loop at all.
Mask multiply runs as one [128,2048] bf16 tensor_tensor per HEAD PAIR
(adjacency duplicated along the free dim), all on DVE (gpsimd contends for
SBUF ports and slows DVE ~4x -- measured).  AV matmul in bf16 (1 cycle/row)
with a ones column appended to the lhsT so row 64 of the accumulator is the
softmax denominator.  Output staged per row-block and stored in 8 batched
DMAs.

Host-side prep (layout/dtype only): h.T, adj.T as bf16, W and W@A as bf16.
"""
import sys

sys.path.insert(0, "/opt/trn_rl_repo")

import numpy as np
import ml_dtypes

import concourse.bass as bass
import concourse.mybir as mybir
import concourse.tile as tile
from concourse.bass_utils import run_bass_kernel_spmd
from concourse.masks import make_identity

F32 = mybir.dt.float32
BF16 = mybir.dt.bfloat16
AF = mybir.ActivationFunctionType
ALU = mybir.AluOpType
BF16NP = ml_dtypes.bfloat16

N_CORES = 8
N = 1024
NB = 8          # row blocks of 128
FIN = 256
KT = 2          # FIN / 128
FO = 512        # heads * fo
H = 8
FOH = 64
ALPHA = 0.2

NSEL = 2        # heads broadcast via PE selector; rest via DMA doubling

_MAX_SYNC_WAITS = 1


def _split_sync_waits(nc, max_waits=_MAX_SYNC_WAITS):
    """This walrus build rejects instructions carrying more than one sync
    wait; hoist extras onto NOPs inserted just before, on the same engine."""
    uid = 0
    for f in nc.m.functions:
        for bb in f.blocks:
            out = []
            for inst in bb.instructions:
                si = getattr(inst, "sync_info", None)
                if si is not None and si.on_wait and len(si.on_wait) > max_waits:
                    waits = list(si.on_wait)
                    keep = waits[-max_waits:]
                    extra = waits[:-max_waits]
                    si.on_wait.clear()
                    si.on_wait.extend(keep)
                    while extra:
                        chunk, extra = extra[:max_waits], extra[max_waits:]
                        nop = mybir.InstNoOp(
                            name=f"waitsplit-{uid}",
                            engine=inst.engine,
                            sync_info=mybir.SyncInfo(
                                on_wait=list(chunk), on_update=[]
                            ),
                            bass_nofuse=True,
                        )
                        uid += 1
                        out.append(nop)
                out.append(inst)
            bb.instructions[:] = out


def build_nc(split=True):
    nc = bass.Bass()
    hT_d = nc.declare_dram_parameter("hTb", [FIN, N], BF16, isOutput=False)
    adjT_d = nc.declare_dram_parameter("adjT", [N, N], BF16, isOutput=False)
    w_d = nc.declare_dram_parameter("Wb", [FIN, FO], BF16, isOutput=False)
    wa_d = nc.declare_dram_parameter("WAb", [FIN, 2 * H], BF16, isOutput=False)
    out_d = nc.declare_dram_parameter("out", [N, FO], F32, isOutput=True)

    with tile.TileContext(nc) as tc:
        with (
            tc.tile_pool(name="const", bufs=1) as const,
            tc.tile_pool(name="persist", bufs=1) as persist,
            tc.tile_pool(name="tp8", bufs=5) as tpool,
            tc.tile_pool(name="epi", bufs=3) as epi,
            tc.tile_pool(name="psS", bufs=3, space="PSUM") as psS,
            tc.tile_pool(name="psAcc", bufs=1, space="PSUM") as psAcc,
        ):
            ident = const.tile([128, 128], F32, tag="ident")
            make_identity(nc, ident[:])

            wk = []
            for k in range(KT):
                t = const.tile([128, FO], BF16, tag=f"W{k}", name=f"W{k}")
                nc.sync.dma_start(t[:], w_d[k * 128:(k + 1) * 128, :])
                wk.append(t)
            wa = []
            for k in range(KT):
                t = const.tile([128, 2 * H], BF16, tag=f"WA{k}", name=f"WA{k}")
                nc.sync.dma_start(t[:], wa_d[k * 128:(k + 1) * 128, :])
                wa.append(t)
            hT = [persist.tile([128, N], BF16, tag=f"hT{k}", name=f"hT{k}")
                  for k in range(KT)]
            for k in range(KT):
                nc.sync.dma_start(hT[k][:], hT_d[k * 128:(k + 1) * 128, :])
            # adjacency duplicated along free dim: adjT2[jb][:, c*N:(c+1)*N]
            # both hold adjT rows jb*128..+128, so one TT masks a head pair
            adjT2 = [persist.tile([128, 2 * N], BF16, tag=f"adjT{j}",
                                  name=f"adjT{j}")
                     for j in range(NB)]
            for jb in range(NB):
                for c in range(2):
                    nc.sync.dma_start(
                        adjT2[jb][:, c * N:(c + 1) * N],
                        adjT_d[jb * 128:(jb + 1) * 128, :],
                    )

            # ---- E_T[16, i] = (WA.T @ hT): rows 0..7 e_src, 8..15 e_dst ----
            e_t = const.tile([16, N], F32, tag="eT")
            for c in range(2):
                ps = psS.tile([16, 512], F32, tag="ps")
                for k in range(KT):
                    nc.tensor.matmul(
                        ps[:], wa[k][:], hT[k][:, c * 512:(c + 1) * 512],
                        start=(k == 0), stop=(k == KT - 1),
                    )
                nc.vector.tensor_copy(e_t[:, c * 512:(c + 1) * 512], ps[:])

            # ---- e_sb[jb][p, 16] = E_T[:, jb*128+p]; s0/s1 = per-j scalars ----
            e_sb = [persist.tile([128, 16], F32, tag=f"E{j}", name=f"E{j}")
                    for j in range(NB)]
            s0sb = [persist.tile([128, H], F32, tag=f"s0{j}", name=f"s0{j}")
                    for j in range(NB)]
            s1sb = [persist.tile([128, H], F32, tag=f"s1{j}", name=f"s1{j}")
                    for j in range(NB)]
            for jb in range(NB):
                tp = psS.tile([128, 512], F32, tag="ps")
                nc.tensor.transpose(
                    tp[:, 0:16], e_t[:, jb * 128:(jb + 1) * 128],
                    ident[0:16, 0:16],
                )
                nc.vector.tensor_copy(e_sb[jb][:], tp[:, 0:16])
                # s0 = exp(alpha * e_dst), s1 = exp(e_dst)
                nc.scalar.activation(
                    s0sb[jb][:], e_sb[jb][:, 8:16], AF.Exp, scale=ALPHA,
                )
                nc.scalar.activation(
                    s1sb[jb][:], e_sb[jb][:, 8:16], AF.Exp, scale=1.0,
                )

            # ---- G8[h, i] = exp(-(1-alpha) * e_src[h, i]) as bf16 ----
            g8 = const.tile([8, N], BF16, tag="g8")
            nc.scalar.activation(g8[:], e_t[0:8, :], AF.Exp, scale=-(1.0 - ALPHA))

            # ---- Gb broadcast over partitions.  Heads 0..1 via PE selector
            # (low latency, unblocks head pair 0); heads 2..7 concatenated on
            # one partition row then log-doubled as a single wide block. ----
            gbsel = [persist.tile([128, N], BF16, tag=f"gb{hh}", name=f"gb{hh}")
                     for hh in range(NSEL)]
            sel = []
            for hh in range(NSEL):
                t = const.tile([8, 128], BF16, tag=f"sel{hh}", name=f"sel{hh}")
                nc.gpsimd.memset(t[:], 0.0)
                nc.gpsimd.affine_select(
                    out=t[:], in_=t[:], pattern=[[0, 128]],
                    compare_op=ALU.not_equal, fill=1.0,
                    base=-hh, channel_multiplier=1,
                )
                sel.append(t)
            for hh in range(NSEL):
                for c in range(2):
                    ps = psS.tile([128, 512], F32, tag="ps")
                    nc.tensor.matmul(
                        ps[:], sel[hh][:], g8[:, c * 512:(c + 1) * 512],
                        start=True, stop=True,
                    )
                    nc.scalar.copy(
                        gbsel[hh][:, c * 512:(c + 1) * 512], ps[:]
                    )
            gball = persist.tile([128, (H - NSEL) * N], BF16, tag="gball")
            for hh in range(NSEL, H):
                nc.sync.dma_start(
                    gball[0:1, (hh - NSEL) * N:(hh - NSEL + 1) * N],
                    g8[hh:hh + 1, :],
                )
            p = 1
            while p < 128:
                nc.sync.dma_start(gball[p:2 * p, :], gball[0:p, :])
                p *= 2

            def gb(hh):
                if hh < NSEL:
                    return gbsel[hh][:, :]
                return gball[:, (hh - NSEL) * N:(hh - NSEL + 1) * N]

            # ---- wh_aug[jb][j, h, 0:64] = (h @ W) block bf16, [:, h, 64] = 1 ----
            wh_aug = [persist.tile([128, H, 65], BF16, tag=f"wha{j}",
                                   name=f"wha{j}")
                      for j in range(NB)]
            for jb in range(NB):
                ps = psS.tile([128, H, FOH], F32, tag="ps")
                for k in range(KT):
                    nc.tensor.matmul(
                        ps[:, :, :], hT[k][:, jb * 128:(jb + 1) * 128], wk[k][:],
                        start=(k == 0), stop=(k == KT - 1),
                    )
                nc.scalar.activation(
                    wh_aug[jb][:, :, 0:64], ps[:, :, :], AF.Copy,
                )
                nc.gpsimd.memset(wh_aug[jb][:, :, 64:65], 1.0)

            # ---- output staging: osm_all[cb][p, h*64+f] ----
            osm_all = [persist.tile([128, FO], F32, tag=f"osm{c}",
                                    name=f"osm{c}")
                       for c in range(NB)]

            # ---- main attention loop, head pairs ----
            for hp in range(H // 2):
                h0, h1 = 2 * hp, 2 * hp + 1
                acc = {
                    (hh, c): psAcc.tile([65, 512], F32, tag=f"acc{hh % 2}{c}",
                                        name=f"acc{hh % 2}{c}")
                    for hh in (h0, h1) for c in range(2)
                }
                for jb in range(NB):
                    t2 = tpool.tile([128, 2 * N], BF16, tag="t2")
                    for q, hh in enumerate((h0, h1)):
                        nc.vector.tensor_scalar(
                            t2[:, q * N:(q + 1) * N], gb(hh),
                            s0sb[jb][:, hh:hh + 1], s1sb[jb][:, hh:hh + 1],
                            ALU.mult, ALU.max,
                        )
                    nc.vector.tensor_mul(t2[:], t2[:], adjT2[jb][:])
                    for q, hh in enumerate((h0, h1)):
                        for c in range(2):
                            nc.tensor.matmul(
                                acc[(hh, c)][:],
                                wh_aug[jb][:, hh, :],
                                t2[:, q * N + c * 512:q * N + (c + 1) * 512],
                                start=(jb == 0), stop=(jb == NB - 1),
                            )
                # epilogue: acc -> SBUF (ACT), transpose back, divide, stage
                for hh in (h0, h1):
                    acc_sb = epi.tile([65, N], F32, tag="accsb")
                    for c in range(2):
                        nc.scalar.copy(
                            acc_sb[:, c * 512:(c + 1) * 512], acc[(hh, c)][:]
                        )
                    rec8 = epi.tile([128, 8], F32, tag="rec8")
                    tps = []
                    for q in range(2):
                        tp = psS.tile([128, 4 * 65], F32, tag="ps")
                        for r in range(4):
                            cb = q * 4 + r
                            nc.tensor.transpose(
                                tp[:, r * 65:r * 65 + 65],
                                acc_sb[:, cb * 128:(cb + 1) * 128],
                                ident[0:65, 0:65],
                            )
                        nc.vector.reciprocal(
                            rec8[:, q * 4:(q + 1) * 4], tp[:, 64::65]
                        )
                        tps.append(tp)
                    for cb in range(NB):
                        q, r = divmod(cb, 4)
                        nc.scalar.activation(
                            osm_all[cb][:, hh * FOH:(hh + 1) * FOH],
                            tps[q][:, r * 65:r * 65 + 64], AF.Copy,
                            scale=rec8[:, cb:cb + 1],
                        )
            for cb in range(NB):
                nc.sync.dma_start(
                    out_d[cb * 128:(cb + 1) * 128, :], osm_all[cb][:]
                )

    if split:
        _split_sync_waits(nc)
    return nc


_NC_CACHE = None


def _get_nc():
    global _NC_CACHE
    if _NC_CACHE is None:
        _NC_CACHE = build_nc()
    return _NC_CACHE


def _prep_in_maps(h, adj, W, a):
    h = np.ascontiguousarray(h, dtype=np.float32)
    adj = np.ascontiguousarray(adj, dtype=np.int32)
    W = np.ascontiguousarray(W, dtype=np.float32)
    a = np.ascontiguousarray(a, dtype=np.float32)
    amat = np.zeros((FO, 2 * H), dtype=np.float32)
    for hh in range(H):
        amat[hh * FOH:(hh + 1) * FOH, hh] = a[hh, :FOH]
        amat[hh * FOH:(hh + 1) * FOH, H + hh] = a[hh, FOH:]
    wamat = (W @ amat).astype(BF16NP)
    wb = W.astype(BF16NP)
    return [
        {
            "hTb": np.ascontiguousarray(h[c].T).astype(BF16NP),
            "adjT": np.ascontiguousarray(adj[c].T).astype(BF16NP),
            "Wb": wb,
            "WAb": wamat,
        }
        for c in range(N_CORES)
    ]


def run(h, adj, W, a, trace=False, **kw):
    nc = _get_nc()
    in_maps = _prep_in_maps(h, adj, W, a)
    res = run_bass_kernel_spmd(nc, in_maps, list(range(N_CORES)), trace=trace, **kw)
    out = np.stack([res.results[c]["out"] for c in range(N_CORES)], axis=0)
    return out.astype(np.float32), res


def kernel(h, adj, W, a):
    out, _ = run(h, adj, W, a)
    return out


# revision 7
# speedup vs baseline: 1.9804x; 1.0464x over previous
"""MultiHeadGAT layer on 8 trn2 NeuronCores, data-parallel over batch.

Per core (one batch element), with softmax-invariant rescaling: dividing the
unnormalized attention P[j,i] = exp(leaky_relu(e_src[i]+e_dst[j])) by
exp(e_src[i]) (a per-i factor that cancels in the softmax) gives

  P'[j,i] = max( exp(-0.8*e_src[i]) * exp(0.2*e_dst[j]),  exp(e_dst[j]) )

i.e. ONE fused DVE tensor_scalar op per [128,1024] tile (mult + max against
two per-partition scalars) in bf16 -- no exps in the main loop at all.
Mask multiply runs as one [128,2048] bf16 tensor_tensor per HEAD PAIR
(adjacency duplicated along the free dim), all on DVE (gpsimd contends for
SBUF ports and slows DVE ~4x -- measured).  AV matmul in bf16 (1 cycle/row)
with a ones column appended to the lhsT so row 64 of the accumulator is the
softmax denominator.  Output staged per row-block and stored in 8 batched
DMAs.

Host-side prep (layout/dtype only): h.T, adj.T as bf16, W and W@A as bf16.
"""
import sys

sys.path.insert(0, "/opt/trn_rl_repo")

import numpy as np
import ml_dtypes

import concourse.bass as bass
import concourse.mybir as mybir
import concourse.tile as tile
from concourse.bass_utils import run_bass_kernel_spmd
from concourse.masks import make_identity

F32 = mybir.dt.float32
BF16 = mybir.dt.bfloat16
AF = mybir.ActivationFunctionType
ALU = mybir.AluOpType
BF16NP = ml_dtypes.bfloat16

N_CORES = 8
N = 1024
NB = 8          # row blocks of 128
FIN = 256
KT = 2          # FIN / 128
FO = 512        # heads * fo
H = 8
FOH = 64
ALPHA = 0.2

NSEL = 4        # heads broadcast via PE selector; rest via DMA doubling

_MAX_SYNC_WAITS = 1


def _split_sync_waits(nc, max_waits=_MAX_SYNC_WAITS):
    """This walrus build rejects instructions carrying more than one sync
    wait; hoist extras onto NOPs inserted just before, on the same engine."""
    uid = 0
    for f in nc.m.functions:
        for bb in f.blocks:
            out = []
            for inst in bb.instructions:
                si = getattr(inst, "sync_info", None)
                if si is not None and si.on_wait and len(si.on_wait) > max_waits:
                    waits = list(si.on_wait)
                    keep = waits[-max_waits:]
                    extra = waits[:-max_waits]
                    si.on_wait.clear()
                    si.on_wait.extend(keep)
                    while extra:
                        chunk, extra = extra[:max_waits], extra[max_waits:]
                        nop = mybir.InstNoOp(
                            name=f"waitsplit-{uid}",
                            engine=inst.engine,
                            sync_info=mybir.SyncInfo(
                                on_wait=list(chunk), on_update=[]
                            ),
                            bass_nofuse=True,
                        )
                        uid += 1
                        out.append(nop)
                out.append(inst)
            bb.instructions[:] = out


def build_nc(split=True):
    nc = bass.Bass()
    hT_d = nc.declare_dram_parameter("hTb", [FIN, N], BF16, isOutput=False)
    adjT_d = nc.declare_dram_parameter("adjT2", [N, 2 * N], BF16, isOutput=False)
    w_d = nc.declare_dram_parameter("Wb", [FIN, FO], BF16, isOutput=False)
    wa_d = nc.declare_dram_parameter("WAb", [FIN, 2 * H], BF16, isOutput=False)
    out_d = nc.declare_dram_parameter("out", [N, FO], F32, isOutput=True)

    with tile.TileContext(nc) as tc:
        with (
            tc.tile_pool(name="const", bufs=1) as const,
            tc.tile_pool(name="persist", bufs=1) as persist,
            tc.tile_pool(name="tp8", bufs=5) as tpool,
            tc.tile_pool(name="epi", bufs=3) as epi,
            tc.tile_pool(name="psS", bufs=3, space="PSUM") as psS,
            tc.tile_pool(name="psAcc", bufs=1, space="PSUM") as psAcc,
        ):
            ident = const.tile([128, 128], F32, tag="ident")
            make_identity(nc, ident[:])

            wa = []
            for k in range(KT):
                t = const.tile([128, 2 * H], BF16, tag=f"WA{k}", name=f"WA{k}")
                nc.sync.dma_start(t[:], wa_d[k * 128:(k + 1) * 128, :])
                wa.append(t)
            hT = [persist.tile([128, N], BF16, tag=f"hT{k}", name=f"hT{k}")
                  for k in range(KT)]
            for k in range(KT):
                nc.sync.dma_start(hT[k][:], hT_d[k * 128:(k + 1) * 128, :])
            wk = []
            for k in range(KT):
                t = const.tile([128, FO], BF16, tag=f"W{k}", name=f"W{k}")
                nc.sync.dma_start(t[:], w_d[k * 128:(k + 1) * 128, :])
                wk.append(t)
            # adjacency duplicated along free dim (host-prepped): one TT
            # masks a head pair
            adjT2 = [persist.tile([128, 2 * N], BF16, tag=f"adjT{j}",
                                  name=f"adjT{j}")
                     for j in range(NB)]
            for jb in range(NB):
                nc.sync.dma_start(
                    adjT2[jb][:], adjT_d[jb * 128:(jb + 1) * 128, :]
                )

            # ---- E_T[16, i] = (WA.T @ hT): rows 0..7 e_src, 8..15 e_dst ----
            e_t = const.tile([16, N], F32, tag="eT")
            for c in range(2):
                ps = psS.tile([16, 512], F32, tag="ps")
                for k in range(KT):
                    nc.tensor.matmul(
                        ps[:], wa[k][:], hT[k][:, c * 512:(c + 1) * 512],
                        start=(k == 0), stop=(k == KT - 1),
                    )
                nc.vector.tensor_copy(e_t[:, c * 512:(c + 1) * 512], ps[:])

            # ---- e_sb[jb][p, 16] = E_T[:, jb*128+p]; s0/s1 = per-j scalars ----
            e_sb = [persist.tile([128, 16], F32, tag=f"E{j}", name=f"E{j}")
                    for j in range(NB)]
            s0sb = [persist.tile([128, H], F32, tag=f"s0{j}", name=f"s0{j}")
                    for j in range(NB)]
            s1sb = [persist.tile([128, H], F32, tag=f"s1{j}", name=f"s1{j}")
                    for j in range(NB)]
            for jb in range(NB):
                tp = psS.tile([128, 512], F32, tag="ps")
                nc.tensor.transpose(
                    tp[:, 0:16], e_t[:, jb * 128:(jb + 1) * 128],
                    ident[0:16, 0:16],
                )
                nc.vector.tensor_copy(e_sb[jb][:], tp[:, 0:16])
                # s0 = exp(alpha * e_dst), s1 = exp(e_dst)
                nc.scalar.activation(
                    s0sb[jb][:], e_sb[jb][:, 8:16], AF.Exp, scale=ALPHA,
                )
                nc.scalar.activation(
                    s1sb[jb][:], e_sb[jb][:, 8:16], AF.Exp, scale=1.0,
                )

            # ---- G8[h, i] = exp(-(1-alpha) * e_src[h, i]) as bf16 ----
            g8 = const.tile([8, N], BF16, tag="g8")
            nc.scalar.activation(g8[:], e_t[0:8, :], AF.Exp, scale=-(1.0 - ALPHA))

            # ---- Gb broadcast over partitions.  Heads 0..1 via PE selector
            # (low latency, unblocks head pair 0); heads 2..7 concatenated on
            # one partition row then log-doubled as a single wide block. ----
            gbsel = [persist.tile([128, N], BF16, tag=f"gb{hh}", name=f"gb{hh}")
                     for hh in range(NSEL)]
            sel = []
            for hh in range(NSEL):
                t = const.tile([8, 128], BF16, tag=f"sel{hh}", name=f"sel{hh}")
                nc.gpsimd.memset(t[:], 0.0)
                nc.gpsimd.affine_select(
                    out=t[:], in_=t[:], pattern=[[0, 128]],
                    compare_op=ALU.not_equal, fill=1.0,
                    base=-hh, channel_multiplier=1,
                )
                sel.append(t)
            for hh in range(NSEL):
                for c in range(2):
                    ps = psS.tile([128, 512], F32, tag="ps")
                    nc.tensor.matmul(
                        ps[:], sel[hh][:], g8[:, c * 512:(c + 1) * 512],
                        start=True, stop=True,
                    )
                    nc.scalar.copy(
                        gbsel[hh][:, c * 512:(c + 1) * 512], ps[:]
                    )
            gball = persist.tile([128, (H - NSEL) * N], BF16, tag="gball")
            for hh in range(NSEL, H):
                nc.sync.dma_start(
                    gball[0:1, (hh - NSEL) * N:(hh - NSEL + 1) * N],
                    g8[hh:hh + 1, :],
                )
            p = 1
            while p < 128:
                nc.sync.dma_start(gball[p:2 * p, :], gball[0:p, :])
                p *= 2

            def gb(hh):
                if hh < NSEL:
                    return gbsel[hh][:, :]
                return gball[:, (hh - NSEL) * N:(hh - NSEL + 1) * N]

            # ---- wh_aug[jb][j, h, 0:64] = (h @ W) block bf16, [:, h, 64] = 1 ----
            wh_aug = [persist.tile([128, H, 65], BF16, tag=f"wha{j}",
                                   name=f"wha{j}")
                      for j in range(NB)]
            for jb in range(NB):
                ps = psS.tile([128, H, FOH], F32, tag="ps")
                for k in range(KT):
                    nc.tensor.matmul(
                        ps[:, :, :], hT[k][:, jb * 128:(jb + 1) * 128], wk[k][:],
                        start=(k == 0), stop=(k == KT - 1),
                    )
                nc.scalar.activation(
                    wh_aug[jb][:, :, 0:64], ps[:, :, :], AF.Copy,
                )
                nc.gpsimd.memset(wh_aug[jb][:, :, 64:65], 1.0)

            # ---- output staging: osm_all[cb][p, h*64+f] ----
            osm_all = [persist.tile([128, FO], F32, tag=f"osm{c}",
                                    name=f"osm{c}")
                       for c in range(NB)]

            # ---- main attention loop, head pairs ----
            for hp in range(H // 2):
                h0, h1 = 2 * hp, 2 * hp + 1
                acc = {
                    (hh, c): psAcc.tile([65, 512], F32, tag=f"acc{hh % 2}{c}",
                                        name=f"acc{hh % 2}{c}")
                    for hh in (h0, h1) for c in range(2)
                }
                for jb in range(NB):
                    t2 = tpool.tile([128, 2 * N], BF16, tag="t2")
                    for q, hh in enumerate((h0, h1)):
                        nc.vector.tensor_scalar(
                            t2[:, q * N:(q + 1) * N], gb(hh),
                            s0sb[jb][:, hh:hh + 1], s1sb[jb][:, hh:hh + 1],
                            ALU.mult, ALU.max,
                        )
                    nc.vector.tensor_mul(t2[:], t2[:], adjT2[jb][:])
                    for q, hh in enumerate((h0, h1)):
                        for c in range(2):
                            nc.tensor.matmul(
                                acc[(hh, c)][:],
                                wh_aug[jb][:, hh, :],
                                t2[:, q * N + c * 512:q * N + (c + 1) * 512],
                                start=(jb == 0), stop=(jb == NB - 1),
                            )
                # epilogue: acc -> SBUF (ACT), transpose back, divide, stage
                for hh in (h0, h1):
                    acc_sb = epi.tile([65, N], F32, tag="accsb")
                    rec8 = epi.tile([128, 8], F32, tag="rec8")
                    for q in range(2):
                        nc.scalar.copy(
                            acc_sb[:, q * 512:(q + 1) * 512], acc[(hh, q)][:]
                        )
                        tp = psS.tile([128, 4 * 65], F32, tag="ps")
                        for r in range(4):
                            cb = q * 4 + r
                            nc.tensor.transpose(
                                tp[:, r * 65:r * 65 + 65],
                                acc_sb[:, cb * 128:(cb + 1) * 128],
                                ident[0:65, 0:65],
                            )
                        nc.vector.reciprocal(
                            rec8[:, q * 4:(q + 1) * 4], tp[:, 64::65]
                        )
                        for r in range(4):
                            cb = q * 4 + r
                            nc.scalar.activation(
                                osm_all[cb][:, hh * FOH:(hh + 1) * FOH],
                                tp[:, r * 65:r * 65 + 64], AF.Copy,
                                scale=rec8[:, cb:cb + 1],
                            )
                # flush finished column halves: heads 0-3 after pair 1,
                # heads 4-7 after pair 3
                if hp in (1, H // 2 - 1):
                    half = 0 if hp == 1 else 1
                    for cb in range(NB):
                        nc.sync.dma_start(
                            out_d[cb * 128:(cb + 1) * 128,
                                  half * 256:(half + 1) * 256],
                            osm_all[cb][:, half * 256:(half + 1) * 256],
                        )

    if split:
        _split_sync_waits(nc)
    return nc


_NC_CACHE = None


def _get_nc():
    global _NC_CACHE
    if _NC_CACHE is None:
        _NC_CACHE = build_nc()
    return _NC_CACHE


def _dup_adjT(adj_c):
    at = np.ascontiguousarray(adj_c.T).astype(BF16NP)
    return np.ascontiguousarray(np.concatenate([at, at], axis=1))


def _prep_in_maps(h, adj, W, a):
    h = np.ascontiguousarray(h, dtype=np.float32)
    adj = np.ascontiguousarray(adj, dtype=np.int32)
    W = np.ascontiguousarray(W, dtype=np.float32)
    a = np.ascontiguousarray(a, dtype=np.float32)
    amat = np.zeros((FO, 2 * H), dtype=np.float32)
    for hh in range(H):
        amat[hh * FOH:(hh + 1) * FOH, hh] = a[hh, :FOH]
        amat[hh * FOH:(hh + 1) * FOH, H + hh] = a[hh, FOH:]
    wamat = (W @ amat).astype(BF16NP)
    wb = W.astype(BF16NP)
    return [
        {
            "hTb": np.ascontiguousarray(h[c].T).astype(BF16NP),
            "adjT2": _dup_adjT(adj[c]),
            "Wb": wb,
            "WAb": wamat,
        }
        for c in range(N_CORES)
    ]


def run(h, adj, W, a, trace=False, **kw):
    nc = _get_nc()
    in_maps = _prep_in_maps(h, adj, W, a)
    res = run_bass_kernel_spmd(nc, in_maps, list(range(N_CORES)), trace=trace, **kw)
    out = np.stack([res.results[c]["out"] for c in range(N_CORES)], axis=0)
    return out.astype(np.float32), res


def kernel(h, adj, W, a):
    out, _ = run(h, adj, W, a)
    return out
